# revision 1
# baseline (speedup 1.0000x reference)
"""Trainium2 Bass kernel for a 4-head attention layer with post-softmax
affine blend (attn = 0.5*softmax(qk/sqrt(dh)) + 0.5), distributed over 8
NeuronCores.

Reference computation (B=2, S=4096, D=128, H=4, Dh=32):
    k = einsum('ihd,bpd->biph', W_K, x)
    q = einsum('ihd,bpd->biph', W_Q, x)
    v = einsum('ihd,bpd->biph', W_V, x)
    scores = einsum('biph,biqh->biqp', k, q) / sqrt(32)
    attn   = softmax(scores, -1) * 0.5 + 0.5
    z      = einsum('biph,biqp->biqh', v, attn)
    out    = einsum('df,bpf->bpd', W_O, z_flat)

Sharding: 8 cores = (batch b in {0,1}) x (query chunk qc in 4 x 1024).
Each core computes all 4 heads for its 1024 queries against all 4096
keys and emits the disjoint output slice out[b, qc*1024:(qc+1)*1024, :].

Per-core algorithm (everything stays on-chip):
  - x^T (transposed on host) is DMA'd in bf16; k^T/q^T/v projections run
    on the TensorEngine with the head dim stacked so that head i's
    k^T/q^T rows live at SBUF partitions 32i..32i+32.
  - scores^T tiles [128 keys x 512 q] are built per (key-block, q-half)
    with 4 row-packed K=32 matmuls (one per head) into one 4-bank PSUM
    tile; ScalarE applies exp (scale folded into W_Q) writing bf16 to
    SBUF.
  - attn@v accumulates z^T[f, q] in PSUM over key-blocks with col-packed
    M=64 matmuls whose stationary operand is [v_i | ones | zero-pad], so
    the softmax denominator accumulates in the same PSUM tile for free.
  - The uniform 0.5*sum_k(v) blend term is folded into a host-computed
    per-batch constant c and added via a K=1 matmul; softmax
    normalization (1/denom) is applied to z^T by VectorE before the
    final W_O projection.
"""

import math

import numpy as np
import ml_dtypes

BF16 = ml_dtypes.bfloat16

B, S, D, H, DH = 2, 4096, 128, 4, 32
QCHUNK = 1024  # queries per core
NCORES = 8
NKB = S // 128  # 32 key blocks
# exp(s) is computed as exp((s * 2^15 * log2(e)) * ln(2) / 2^15); the big
# pre-scale is folded into W_Q so a bit-trick exp2 on VectorE can share the
# same score tensor later.
PRESCALE = (2.0**15) * math.log2(math.e) / math.sqrt(DH)
ACT_SCALE = math.log(2.0) / (2.0**15)

# Schraudolph exp2 constant: sigma balances the multiplicative error of the
# linear-mantissa approximation; folded into the int16 bf16-bit construction.
EXP2_SIGMA = 0.02979

_PROGRAM = None


def _register_exp2():
    """Register (once) a fused y = x*C0 + C1 custom DVE op whose int16
    output, reinterpreted as bf16, is 2^(x/2^15) a la Schraudolph."""
    from concourse import dve_ops
    from concourse.dve_spec import Spec, Src0, C0, C1, lower, _has_src1
    from concourse.dve_uop import DveOpSpec

    name = "EXP2_SCHRAU_ANT"
    for o in dve_ops.OPS:
        if o.name == name:
            return o
    spec = Spec(body=Src0 * C0 + C1,
                reference=lambda in0, in1, c0, c1, c2: in0 * c0 + c1)
    opcode = dve_ops._CUSTOM_DVE_ROW_BASE + len(dve_ops.OPS)
    shas = {}
    for ver in ("v3", "v4"):
        s = DveOpSpec(name=name, opcode=opcode, uops=lower(spec, ver=ver),
                      rd1_en=_has_src1(spec))
        shas[ver] = s.sha(ver)
    op = dve_ops.DveOp(name, spec, subdim=False, uops_sha=shas)
    dve_ops.OPS.append(op)
    dve_ops.CUSTOM_DVE_SPECS[name] = spec
    dve_ops._SUB_OPCODE_FOR_NAME[name] = opcode
    return op


def _build_program(loop_n: int = 1, exp_mode: str = "act16"):
    import concourse.bass as bass
    import concourse.mybir as mybir
    import concourse.tile as tile
    from concourse import bacc
    from contextlib import ExitStack

    import dataclasses

    f32 = mybir.dt.float32
    bf16 = mybir.dt.bfloat16
    AF = mybir.ActivationFunctionType
    exp2_op = _register_exp2()

    def i16_alias(ap):
        h = dataclasses.replace(ap.tensor, dtype=mybir.dt.int16)
        return bass.AP(tensor=h, offset=ap.offset, ap=[list(d) for d in ap.ap])

    def bf16_hi_alias(ap):
        """View an f32 [P, N] AP as the bf16 high halves: [P, N] bf16,
        element stride 2, offset +1 (little-endian high 2 bytes)."""
        h = dataclasses.replace(
            ap.tensor, dtype=mybir.dt.bfloat16,
            shape=[ap.tensor.shape[0], ap.tensor.shape[1] * 2],
        )
        newap = [[ap.ap[0][0] * 2, ap.ap[0][1]]] + [
            [d[0] * 2, d[1]] for d in ap.ap[1:]
        ]
        return bass.AP(tensor=h, offset=ap.offset * 2 + 1, ap=newap)

    nc = bacc.Bacc(None, target_bir_lowering=False)

    xkT = nc.dram_tensor("xkT", [D, S], bf16, kind="ExternalInput")
    xqT = nc.dram_tensor("xqT", [D, QCHUNK], bf16, kind="ExternalInput")
    wqT = nc.dram_tensor("wqT", [D, H * DH], bf16, kind="ExternalInput")
    wkT = nc.dram_tensor("wkT", [D, H * DH], bf16, kind="ExternalInput")
    wvT = nc.dram_tensor("wvT", [D, H * DH], bf16, kind="ExternalInput")
    woT = nc.dram_tensor("woT", [2, 128, D], bf16, kind="ExternalInput")
    cvec = nc.dram_tensor("cvec", [1, D], f32, kind="ExternalInput")
    out = nc.dram_tensor("out", [QCHUNK, D], f32, kind="ExternalOutput")

    with tile.TileContext(nc) as tc, ExitStack() as ctx:
        if loop_n > 1:
            ctx.enter_context(tc.For_i(0, loop_n, 1))
        const = ctx.enter_context(tc.tile_pool(name="const", bufs=1))
        work = ctx.enter_context(tc.tile_pool(name="work", bufs=1))

        # ---- constants / persistent SBUF tensors ----
        w_sb = {}
        for name, dram in (("wq", wqT), ("wk", wkT), ("wv", wvT)):
            t = const.tile([128, 128], bf16, tag=f"w_{name}", name=f"w_{name}")
            nc.sync.dma_start(out=t, in_=dram[:, :])
            w_sb[name] = t
        wo_sb = const.tile([128, 2, 128], bf16, tag="wo_sb")
        for p in range(2):
            nc.sync.dma_start(out=wo_sb[:, p, :], in_=woT[p, :, :])
        c_sb = const.tile([1, D], f32, tag="c_sb")
        nc.sync.dma_start(out=c_sb, in_=cvec[:, :])
        ones1 = const.tile([1, 128], f32, tag="ones1")
        nc.vector.memset(ones1, 1.0)
        zrow = const.tile([1, 512], bf16, tag="zrow")
        nc.vector.memset(zrow, 0.0)

        xq_sb = const.tile([128, QCHUNK], bf16, tag="xq_sb")
        nc.sync.dma_start(out=xq_sb, in_=xqT[:, :])
        xk_sb = const.tile([128, S], bf16, tag="xk_sb")
        kT_sb = const.tile([128, S], bf16, tag="kT_sb")
        qT_sb = const.tile([128, QCHUNK], bf16, tag="qT_sb")
        # v_sb[key, kb, head, 0:32]=v, [...,32]=1.0, [...,33:64]=0
        v_sb = const.tile([128, NKB, H, 64], bf16, tag="v_sb")
        nc.gpsimd.memset(v_sb, 0.0)
        nc.gpsimd.memset(v_sb[:, :, :, 32], 1.0)

        # ---- projections (chunks interleave with the first rounds) ----
        proj_ps = ctx.enter_context(tc.tile_pool(name="proj_ps", bufs=2, space="PSUM"))

        def emit_proj_chunk(c8):
            sl = slice(c8 * 512, (c8 + 1) * 512)
            nc.sync.dma_start(out=xk_sb[:, sl], in_=xkT[:, sl])
            pk = proj_ps.tile([128, 512], f32, tag="pj", name="pk")
            nc.tensor.matmul(pk, w_sb["wk"], xk_sb[:, sl], start=True, stop=True)
            nc.vector.tensor_copy(out=kT_sb[:, sl], in_=pk)
            for j in range(4):  # 128-col key blocks inside the chunk
                kb = c8 * 4 + j
                ksl = slice(kb * 128, (kb + 1) * 128)
                pv = proj_ps.tile([128, 512], f32, tag="pj", name="pv")[:, 0:128]
                nc.tensor.matmul(pv, xk_sb[:, ksl], w_sb["wv"], start=True, stop=True)
                # scatter heads into the [head, 64] aug layout
                nc.vector.tensor_copy(
                    out=v_sb[:, kb, :, 0:32],
                    in_=pv.rearrange("p (i h) -> p i h", i=H),
                )

        for qh in range(2):
            sl = slice(qh * 512, (qh + 1) * 512)
            pq = proj_ps.tile([128, 512], f32, tag="pj", name="pq")
            nc.tensor.matmul(pq, w_sb["wq"], xq_sb[:, sl], start=True, stop=True)
            nc.vector.tensor_copy(out=qT_sb[:, sl], in_=pq)

        # ---- main rounds: scores^T -> exp -> z^T accumulation ----
        zden_ps = ctx.enter_context(tc.tile_pool(name="zden_ps", bufs=1, space="PSUM"))
        round_ctx = ExitStack()
        st_ps = round_ctx.enter_context(tc.tile_pool(name="st_ps", bufs=1, space="PSUM"))
        exp_pool = round_ctx.enter_context(tc.tile_pool(name="exp_pool", bufs=2))

        dram_pool = ctx.enter_context(
            tc.tile_pool(name="dram_pool", bufs=1, space="DRAM")
        )
        rec_dram = [
            [dram_pool.tile([1, 512], mybir.dt.float32, tag=f"rd_{p}_{j}_{qh}",
                            name=f"rd_{p}_{j}_{qh}") for j in range(2) for qh in range(2)]
            for p in range(2)
        ]
        rec = [work.tile([128, QCHUNK], f32, tag=f"rec_{p}", name=f"rec_{p}") for p in range(2)]
        rep = [work.tile([128, QCHUNK], f32, tag=f"rep_{p}", name=f"rep_{p}") for p in range(2)]
        zT_sb = [work.tile([128, QCHUNK], bf16, tag=f"zT_{p}", name=f"zT_{p}") for p in range(2)]

        # z/denominator accumulators: [pair] -> [128, 512] for the current
        # q-half; rows 0:32 z of head 2p, row 32 its denom, rows 64:96 z of
        # head 2p+1, row 96 its denom. qh1 reuses qh0's banks (bufs=1 tags)
        # once qh0's normalization has read them.
        z_cur = [None, None]

        def start_qh():
            for p in range(2):
                z_cur[p] = zden_ps.tile(
                    [128, 512], f32, tag=f"z_{p}", name=f"z_{p}"
                )
                nc.tensor.matmul(
                    z_cur[p], zrow[:, 0:128], zrow, start=True, stop=False,
                    skip_group_check=True,
                )
        def emit_round(qh, kb):
            qsl = slice(qh * 512, (qh + 1) * 512)
            if True:
                ksl = slice(kb * 128, (kb + 1) * 128)
                ex = [None, None]
                for p in range(2):
                    st = st_ps.tile([128, 1024], f32, tag=f"st_{p}", name=f"st_{p}")
                    for j in range(2):
                        i = 2 * p + j
                        nc.tensor.matmul(
                            st[:, j * 512 : (j + 1) * 512],
                            kT_sb[32 * i : 32 * (i + 1), ksl],
                            qT_sb[32 * i : 32 * (i + 1), qsl],
                            start=True,
                            stop=True,
                            tile_position=(32 * i, 0),
                        )
                    e = exp_pool.tile([128, 1024], bf16, tag=f"ex_{p}", name=f"ex_{p}")
                    if exp_mode == "mix4" and p == 1 and kb % 2 == 0:
                        nc.vector._custom_dve(
                            exp2_op, out=i16_alias(e[:, :]), in0=st[:, :],
                            s0=1.0 / 256.0, s1=(127.0 - EXP2_SIGMA) * 128.0,
                        )
                    elif exp_mode in ("act16", "mix4"):
                        nc.scalar.activation(
                            out=e, in_=bf16_hi_alias(st[:, :]), func=AF.Exp,
                            scale=ACT_SCALE,
                        )
                    elif p == 0 or exp_mode == "act":
                        nc.scalar.activation(
                            out=e, in_=st, func=AF.Exp, scale=ACT_SCALE
                        )
                    else:
                        nc.vector._custom_dve(
                            exp2_op, out=i16_alias(e[:, :]), in0=st[:, :],
                            s0=1.0 / 256.0, s1=(127.0 - EXP2_SIGMA) * 128.0,
                        )
                    ex[p] = e
                for p in range(2):
                    for j in range(2):
                        nc.tensor.matmul(
                            z_cur[p][64 * j : 64 * j + 64, :],
                            v_sb[:, kb, 2 * p + j, :],
                            ex[p][:, j * 512 : (j + 1) * 512],
                            start=False,
                            stop=(kb == NKB - 1),
                            tile_position=(0, 64 * j),
                            skip_group_check=True,
                        )
        def emit_epilogue(qh):
            # per-qh normalization, overlapped with the next qh's rounds:
            # reciprocal of denominators stays on partitions 32/96, bounces
            # through DRAM, and is partition-broadcast back over the z rows.
            qsl = slice(qh * 512, (qh + 1) * 512)
            for p in range(2):
                for j in range(2):
                    r = 64 * j + 32
                    nc.vector.reciprocal(
                        out=rec[p][r : r + 1, qsl], in_=z_cur[p][r : r + 1, :]
                    )
                    rd = rec_dram[p][2 * j + qh]
                    nc.sync.dma_start(out=rd, in_=rec[p][r : r + 1, qsl])
                    src = rd[0, :]
                    bcast = bass.AP(
                        tensor=src.tensor, offset=src.offset, ap=[[0, 32], [1, 512]]
                    )
                    nc.sync.dma_start(
                        out=rep[p][64 * j : 64 * j + 32, qsl], in_=bcast
                    )
                    rsl = slice(64 * j, 64 * j + 32)
                    nc.vector.tensor_mul(
                        zT_sb[p][rsl, qsl], z_cur[p][rsl, :], rep[p][rsl, qsl]
                    )

        start_qh()
        for c8 in range(8):
            emit_proj_chunk(c8)
            for kb in range(4 * c8, 4 * c8 + 4):
                emit_round(0, kb)
        emit_epilogue(0)
        start_qh()
        for kb in range(NKB):
            emit_round(1, kb)
        emit_epilogue(1)

        round_ctx.close()

        # ---- final projection + blend constant ----
        with tc.tile_pool(name="u_ps", bufs=2, space="PSUM") as u_ps, tc.tile_pool(
            name="out_pool", bufs=2
        ) as out_pool:
            for qb in range(QCHUNK // 128):
                bsl = slice(qb * 128, (qb + 1) * 128)
                ue = u_ps.tile([128, 128], f32, tag="ue")
                uo = u_ps.tile([128, 128], f32, tag="uo")
                # heads 0,2 -> ue (lhsT partitions 0:32); heads 1,3 -> uo (64:96)
                nc.tensor.matmul(
                    ue, zT_sb[0][0:32, bsl], wo_sb[0:32, 0, :], start=True, stop=False,
                    skip_group_check=True, tile_position=(0, 0),
                )
                nc.tensor.matmul(
                    ue, zT_sb[1][0:32, bsl], wo_sb[0:32, 1, :], start=False,
                    stop=False, skip_group_check=True, tile_position=(0, 0),
                )
                nc.tensor.matmul(
                    ue, ones1, c_sb, start=False,
                    stop=True, skip_group_check=True, tile_position=(0, 0),
                )
                nc.tensor.matmul(
                    uo, zT_sb[0][64:96, bsl], wo_sb[64:96, 0, :], start=True,
                    stop=False, skip_group_check=True, tile_position=(64, 0),
                )
                nc.tensor.matmul(
                    uo, zT_sb[1][64:96, bsl], wo_sb[64:96, 1, :], start=False,
                    stop=True, skip_group_check=True, tile_position=(64, 0),
                )
                ob = out_pool.tile([128, 128], f32, tag="ob")
                nc.scalar.copy(out=ob, in_=ue)
                nc.vector.tensor_add(ob, ob, uo)
                nc.sync.dma_start(out=out[bsl, :], in_=ob)

    nc.compile()
    return nc


def _get_program(loop_n: int = 1):
    global _PROGRAM
    if loop_n != 1:
        return _build_program(loop_n)
    if _PROGRAM is None:
        _PROGRAM = _build_program()
    return _PROGRAM


def make_in_maps(x, W_K, W_Q, W_V, W_O):
    x = np.asarray(x, np.float32)
    W_K = np.asarray(W_K, np.float32)
    W_Q = np.asarray(W_Q, np.float32)
    W_V = np.asarray(W_V, np.float32)
    W_O = np.asarray(W_O, np.float32)

    wqT = np.ascontiguousarray((W_Q.transpose(2, 0, 1).reshape(D, H * DH)) * PRESCALE)
    wkT = np.ascontiguousarray(W_K.transpose(2, 0, 1).reshape(D, H * DH))
    wvT = np.ascontiguousarray(W_V.transpose(2, 0, 1).reshape(D, H * DH))
    woT_flat = 0.5 * W_O.T  # [f, d']
    woT = np.zeros((2, 128, D), np.float32)
    for p in range(2):
        woT[p, 0:32] = woT_flat[(2 * p) * 32 : (2 * p) * 32 + 32]
        woT[p, 64:96] = woT_flat[(2 * p + 1) * 32 : (2 * p + 1) * 32 + 32]

    in_maps = []
    for core in range(NCORES):
        b, qc = divmod(core, 4)
        xb = x[b]
        xkT_b = np.ascontiguousarray(xb.T).astype(BF16)
        xqT_c = np.ascontiguousarray(xb[qc * QCHUNK : (qc + 1) * QCHUNK].T).astype(BF16)
        # exact blend constant: c = 0.5 * (sum_k v[k]) @ W_O^T
        sv = (xb.sum(0, dtype=np.float64) @ wvT.astype(np.float64))
        c = 0.5 * (sv @ W_O.T.astype(np.float64))
        in_maps.append(
            {
                "xkT": xkT_b,
                "xqT": xqT_c,
                "wqT": wqT.astype(BF16),
                "wkT": wkT.astype(BF16),
                "wvT": wvT.astype(BF16),
                "woT": woT.astype(BF16),
                "cvec": np.ascontiguousarray(c[None, :]).astype(np.float32),
            }
        )
    return in_maps


def kernel(x, W_K, W_Q, W_V, W_O):
    from concourse.bass_utils import run_bass_kernel_spmd

    nc = _get_program()
    in_maps = make_in_maps(x, W_K, W_Q, W_V, W_O)
    res = run_bass_kernel_spmd(nc, in_maps, core_ids=list(range(NCORES)))
    full = np.empty((B, S, D), np.float32)
    for core in range(NCORES):
        b, qc = divmod(core, 4)
        full[b, qc * QCHUNK : (qc + 1) * QCHUNK, :] = res.results[core]["out"]
    return full

